# revision 18
# baseline (speedup 1.0000x reference)
"""CRF loss (2-state FSA) on 8 Trainium2 NeuronCores — v5.

Math: with y = exp(log_probs), the per-step denominator scores are linear in
y, so the 2-state forward recurrence runs in REAL space as products of 2x2
matrices M_t = [[S0, S1], [e00*p, e11*p]] (p = y[:, 2]), composed on-device
over chunks of LCH=2 steps, scaled by 32/step against fp32 underflow (exact
correction removed in the host fold). Steps past input_len become 32*I.

Device layout (per core, 8 sequences / 32768 positions): the host ships
log-probs TRANSPOSED — classes on the 128 partitions, positions along the
free axis in (j, q) order so that position q*256+j lands in column j*128+q.
PE matmuls over each 128-column block then produce (32*S0, 32*S1, 32*e00*p,
32*e11*p) per position directly in the scan layout: PSUM partition q holds
the 256 consecutive positions of partition q, block index j as the free
axis. No on-device transpose and no DMA-transpose traffic.

Shipping lpT in fp16 halves HBM traffic (the memory bottleneck); the arc
weights ride in the matmul rhs as bf16 hi+lo splits for fp32-equivalent
weight precision.

Numerator: the host re-encodes labels as a one-hot matrix in the same
transposed layout (fp8, 0/1 exact, invalid positions zeroed). The PE
accumulates D += lpT_j^T @ onehot_j over all 256 blocks in one PSUM tile;
diag(D)[q] = sum_j lp[q*256+j, lab] so trace(D) is the core's numerator.

Pipelining: eighth-sized stages; lp DMAs are front-loaded (exp on ACT is
the longest engine chain, it must never starve) while oh DMAs and the
numerator trace matmuls trail two eighths behind so a late oh never stalls
the in-order PE queue ahead of the S-matmuls. Mask+chunk-scan+writeback run
per quarter as soon as that quarter's S-entries land in PSUM; result DMAs
issue from the DVE queue so the SP queue carries only input loads in order.

Host: softmax of the 254 den_scores (constant prep), logs of the shipped
chunk products, the log-space fold per sequence, and num - den.
"""

import os
import sys

import ml_dtypes
import numpy as np

for _p in ("/opt/trn_rl_repo", os.path.expanduser("~/.axon_site/_ro/trn_rl_repo")):
    if os.path.isdir(_p) and _p not in sys.path:
        sys.path.insert(0, _p)

import concourse.bacc as bacc
import concourse.bass as bass
import concourse.mybir as mybir
import concourse.tile as tile
from concourse.bass_utils import run_bass_kernel_spmd

F32 = mybir.dt.float32
F16 = mybir.dt.float16
BF16 = mybir.dt.bfloat16
FP8 = mybir.dt.float8e4
Alu = mybir.AluOpType
Act = mybir.ActivationFunctionType

L = 125
C = 128          # symbol classes
B, T = 64, 4096
NCORES = 8
BSH = B // NCORES            # sequences per core = 8
BT = BSH * T                 # positions per core = 32768
NI = BT // 128               # positions per partition = 256
NE = 8                       # eighths (DMA/compute pipelining)
NIE = NI // NE               # 32 blocks per eighth
NBLK = BT // 128             # 256 column blocks
LCH = 2                      # scan chunk length
NCH = NI // LCH              # 128 chunk matrices per partition
NSEC = 8                     # scan sections (one per eighth)
NIS = NI // NSEC             # 64 positions per section
NCS = NCH // NSEC            # 32 chunks per section
SCALE = 32.0                 # per-step scaling against fp32 underflow
TRACE_LAG = 2                # eighths the numerator matmuls trail behind


def _build_program():
    nc = bacc.Bacc("TRN2", target_bir_lowering=False, debug=False)

    lp_d = nc.dram_tensor("lpt", [128, BT], F16, kind="ExternalInput")
    oh_d = nc.dram_tensor("oh", [128, BT], FP8, kind="ExternalInput")
    u_d = nc.dram_tensor("u16", [128, 8], BF16, kind="ExternalInput")
    mk_d = nc.dram_tensor("mk", [128, 2 * NI], BF16, kind="ExternalInput")
    eye_d = nc.dram_tensor("eye", [128, 128], BF16, kind="ExternalInput")

    chunkp_d = nc.dram_tensor("chunkp", [128, 4, NCH], BF16, kind="ExternalOutput")
    nump_d = nc.dram_tensor("nump", [128, 128], F32, kind="ExternalOutput")

    with tile.TileContext(nc) as tc:
        with (
            tc.tile_pool(name="const", bufs=1) as cpool,
            tc.tile_pool(name="psW", bufs=1, space=bass.MemorySpace.PSUM) as psW,
            tc.tile_pool(name="lp", bufs=1) as lp_pool,
            tc.tile_pool(name="oh", bufs=1) as oh_pool,
            tc.tile_pool(name="y", bufs=1) as y_pool,
            tc.tile_pool(name="sm", bufs=2) as sm_pool,
            tc.tile_pool(name="scan", bufs=2) as scan_pool,
            tc.tile_pool(name="psS", bufs=1, space=bass.MemorySpace.PSUM) as psS,
            tc.tile_pool(name="psD", bufs=1, space=bass.MemorySpace.PSUM) as psD,
        ):
            u16 = cpool.tile([128, 8], BF16)
            pall = cpool.tile([128, 4 * NCH], BF16)
            mk = cpool.tile([128, 2 * NI], BF16)
            eye = cpool.tile([128, 128], BF16)

            # all S-matrix entries for the whole core: [q, (j, 4)]
            sps = psS.tile([128, 4 * NI], F32)
            # numerator trace accumulator
            dps = psD.tile([128, 128], F32)

            lp_tiles = {}
            oh_tiles = {}

            def dma_in(e, half=None):
                if half is None:
                    t = lp_pool.tile([128, NIE * 128], F16, tag=f"lp{e % NE}")
                    nc.sync.dma_start(
                        t[:], lp_d.ap()[:, e * NIE * 128 : (e + 1) * NIE * 128]
                    )
                    lp_tiles[e] = t
                else:
                    # half is a (start, end) column range within the eighth
                    a, b = half
                    if a == 0:
                        tlp = lp_pool.tile(
                            [128, NIE * 128], F16, tag=f"lp{e % NE}"
                        )
                        lp_tiles[e] = tlp
                    t = lp_tiles[e]
                    nc.sync.dma_start(
                        t[:, a:b],
                        lp_d.ap()[:, e * NIE * 128 + a : e * NIE * 128 + b],
                    )

            def dma_oh(e, parts=2):
                toh = oh_pool.tile([128, NIE * 128], FP8, tag=f"oh{e % NE}")
                oh_tiles[e] = toh
                hw = NIE * 128 // parts
                for h in range(parts):
                    nc.sync.dma_start(
                        toh[:, h * hw : (h + 1) * hw],
                        oh_d.ap()[
                            :,
                            e * NIE * 128 + h * hw
                            : e * NIE * 128 + (h + 1) * hw,
                        ],
                    )

            def trace_mm(e):
                lp16, oh8 = lp_tiles[e], oh_tiles[e]
                with tc.high_priority(offset=-100000):
                    for j in range(NIE):
                        J = e * NIE + j
                        nc.tensor.matmul(
                            dps[:],
                            lp16[:, j * 128 : (j + 1) * 128],
                            oh8[:, j * 128 : (j + 1) * 128],
                            start=(J == 0), stop=(J == NBLK - 1),
                            skip_group_check=True,
                        )

            def scan_section(s):
                """Mask+scan+writeback for positions [NIS*s, NIS*(s+1))."""
                j0 = NIS * s
                s4 = sps[:, 4 * j0 : 4 * (j0 + NIS)].rearrange(
                    "p (j four) -> p j four", four=4
                )
                mm_s = mk[:, j0 : j0 + NIS]
                wc_s = mk[:, NI + j0 : NI + j0 + NIS]
                sm0 = sm_pool.tile([128, NIS], F32, tag="sm0")
                nc.vector.tensor_tensor(sm0[:], s4[:, :, 0:1], mm_s, Alu.mult)
                nc.vector.tensor_tensor(sm0[:], sm0[:], wc_s, Alu.add)
                sm1 = sm_pool.tile([128, NIS], F32, tag="sm1")
                nc.vector.tensor_tensor(sm1[:], s4[:, :, 1:2], mm_s, Alu.mult)
                a_t = sm_pool.tile([128, NIS], F32, tag="a_t")
                nc.vector.tensor_tensor(a_t[:], s4[:, :, 2:3], mm_s, Alu.mult)
                b_t = sm_pool.tile([128, NIS], F32, tag="b_t")
                nc.vector.tensor_tensor(b_t[:], s4[:, :, 3:4], mm_s, Alu.mult)
                nc.vector.tensor_tensor(b_t[:], b_t[:], wc_s, Alu.add)

                def step_slice(t_ap, t):
                    return t_ap[:].rearrange("p (c l) -> p c l", l=LCH)[
                        :, :, t : t + 1
                    ]

                # P(chunk) = M_1 @ M_0 — M_0 entries read as strided
                # slices directly (no staging copies); the four output
                # entries' 3-op chains split across Pool and DVE
                out_of = {
                    "00": pall[:, 0 * NCH + NCS * s : 0 * NCH + NCS * (s + 1)],
                    "01": pall[:, 1 * NCH + NCS * s : 1 * NCH + NCS * (s + 1)],
                    "10": pall[:, 2 * NCH + NCS * s : 2 * NCH + NCS * (s + 1)],
                    "11": pall[:, 3 * NCH + NCS * s : 3 * NCH + NCS * (s + 1)],
                }
                s00, s10 = step_slice(sm0, 0), step_slice(sm1, 0)
                a0, b0 = step_slice(a_t, 0), step_slice(b_t, 0)
                s0t, s1t = step_slice(sm0, 1), step_slice(sm1, 1)
                att, btt = step_slice(a_t, 1), step_slice(b_t, 1)
                for col, pc0, pc1 in (("0", s00, a0), ("1", s10, b0)):
                    eng0 = nc.gpsimd if col == "0" else nc.vector
                    eng1 = nc.vector if col == "0" else nc.gpsimd
                    n0 = out_of["0" + col]
                    t1 = scan_pool.tile([128, NCS], F32, tag=f"t1{col}")
                    eng0.tensor_tensor(t1[:], s0t, pc0, Alu.mult)
                    eng0.tensor_tensor(n0, s1t, pc1, Alu.mult)
                    eng0.tensor_tensor(n0, t1[:], n0, Alu.add)
                    n1 = out_of["1" + col]
                    t2 = scan_pool.tile([128, NCS], F32, tag=f"t2{col}")
                    eng1.tensor_tensor(t2[:], att, pc0, Alu.mult)
                    eng1.tensor_tensor(n1, btt, pc1, Alu.mult)
                    eng1.tensor_tensor(n1, t2[:], n1, Alu.add)

            # ---- front-loaded input DMA order (SP queue, in-order) ----
            # lp0 lp1 lp2 lp3 oh0 [u,mm,wc] lp4 oh1 lp5 oh2 lp6 oh3 lp7 ...
            dma_in(0, half=(0, 512))
            nc.sync.dma_start(u16[:], u_d.ap())
            dma_in(0, half=(512, NIE * 128))
            dma_in(1)
            dma_in(2)
            dma_in(3)
            dma_oh(0)
            nc.sync.dma_start(mk[:], mk_d.ap())
            nc.sync.dma_start(eye[:], eye_d.ap())

            for e in range(NE):
                if e + 4 < NE:
                    dma_in(e + 4)
                if 0 < e < 3:
                    dma_oh(e)
                elif 4 <= e <= 5:
                    dma_oh(e - 1)

                lp16 = lp_tiles[e]
                y16 = y_pool.tile([128, NIE * 128], BF16, tag=f"y{e % 3}")
                if e == 0:
                    nc.scalar.activation(y16[:, :512], lp16[:, :512], Act.Exp)
                    nc.scalar.activation(y16[:, 512:], lp16[:, 512:], Act.Exp)
                else:
                    nc.scalar.activation(y16[:], lp16[:], Act.Exp)

                for j in range(NIE):
                    J = e * NIE + j
                    yb = y16[:, j * 128 : (j + 1) * 128]
                    nc.tensor.matmul(
                        sps[:, 4 * J : 4 * J + 4], yb, u16[:, 0:4],
                        start=True, stop=False,
                    )
                    nc.tensor.matmul(
                        sps[:, 4 * J : 4 * J + 4], yb, u16[:, 4:8],
                        start=False, stop=True,
                    )

                if e >= TRACE_LAG:
                    trace_mm(e - TRACE_LAG)
                if e == NE - TRACE_LAG:
                    dma_oh(e - 1)
                    dma_oh(e)
                elif e == NE - 1:
                    dma_oh(e, parts=4)

                scan_section(e)

            trace_mm(NE - TRACE_LAG)
            trace_mm(NE - 1)

            # ---------------- outputs ----------------
            nc.sync.dma_start(
                chunkp_d.ap().rearrange("p e c -> p (e c)"), pall[:]
            )
            ddiag = cpool.tile([128, 128], F32)
            nc.vector.tensor_tensor(ddiag[:], dps[:], eye[:], Alu.mult)
            nc.sync.dma_start(nump_d.ap(), ddiag[:])

    nc.compile()
    return nc


_NC_CACHE = None


def _get_program():
    global _NC_CACHE
    if _NC_CACHE is None:
        _NC_CACHE = _build_program()
    return _NC_CACHE


def _softmax(x):
    x = x.astype(np.float64)
    e = np.exp(x - x.max())
    return e / e.sum()


def _make_in_maps(log_probs, den_scores, input_lens, labels):
    # arc weights: per-state softmax mapped to class columns, pre-scaled by 32
    u0 = _softmax(den_scores[: L + 3])          # [128] state-0 arcs (incl final)
    u1 = _softmax(den_scores[L + 3 :])          # [126] state-1 arcs
    U = np.zeros((128, 4), np.float64)
    U[1, 0] = u0[0]                              # 'O' from state 0
    U[3:128, 0] = u0[1 : L + 1]                  # labels from state 0
    U[3:128, 1] = u1[1 : L + 1]                  # labels from state 1
    U[2, 2] = u0[L + 1]                          # e00: 0 -> 1 emitting I-
    U[2, 3] = u1[0]                              # e11: 1 -> 1 emitting I-
    U *= SCALE
    Uhi = U.astype(ml_dtypes.bfloat16)
    Ulo = (U - Uhi.astype(np.float64)).astype(ml_dtypes.bfloat16)
    u16 = np.ascontiguousarray(
        np.concatenate([Uhi, Ulo], axis=1).astype(ml_dtypes.bfloat16)
    )

    # per-partition length masks: partition q holds positions [256q, 256q+256)
    pids = np.arange(128)
    seq_of_p = pids // 16
    off_of_p = (pids % 16) * NI
    thr = input_lens.reshape(NCORES, BSH)        # [core, seq]
    iota = np.arange(NI)

    lp16 = log_probs.astype(np.float16)          # [B, T, C]
    eye16 = np.ascontiguousarray(np.eye(128, dtype=ml_dtypes.bfloat16))
    one_f8 = np.array(1.0, dtype=ml_dtypes.float8_e4m3)

    in_maps = []
    for k in range(NCORES):
        # transposed, block-permuted log probs: [c, j*128 + q] = lp[q*256+j, c]
        A = lp16[k * BSH : (k + 1) * BSH].reshape(128, NI, C)     # [q, j, c]
        lpt = np.ascontiguousarray(A.transpose(2, 1, 0)).reshape(128, BT)

        labQ = labels[k * BSH : (k + 1) * BSH].reshape(128, NI)   # [q, j]
        thr_k = (thr[k][seq_of_p] - off_of_p)                     # [128]
        validQ = iota[None, :] < thr_k[:, None]                   # [q, j]
        lab_m = np.where(validQ, labQ, -1)
        ohb = (lab_m.T[None, :, :] == np.arange(128)[:, None, None])  # [c, j, q]
        oh8 = (ohb.astype(np.uint8) * one_f8.view(np.uint8)).reshape(128, BT)
        oh8 = oh8.view(ml_dtypes.float8_e4m3)

        mmf = validQ.astype(np.float32)                           # [128, NI]
        mk = np.concatenate([mmf, SCALE * (1.0 - mmf)], axis=1)
        mk = np.ascontiguousarray(mk.astype(ml_dtypes.bfloat16))

        in_maps.append(dict(lpt=lpt, oh=oh8, u16=u16, mk=mk, eye=eye16))
    return in_maps


def _combine_host(results, den_scores):
    """Fold per-core device outputs into the scalar loss (float64 host fold)."""
    s0 = den_scores.astype(np.float64)[: L + 3]
    fs = float(s0[L + 2] - np.log(np.exp(s0 - s0.max()).sum()) - s0.max())
    num = 0.0
    logM_all = []
    corr = LCH * np.log(SCALE)
    for res in results:
        num += float(res["nump"].astype(np.float64).sum())
        with np.errstate(divide="ignore"):
            cl = np.log(res["chunkp"].astype(np.float64))   # [128, 4, NCH]
        # partition q -> (seq_local = q//16, toff = q%16); chunk order (toff, c)
        cl = cl.reshape(BSH, 16, 4, NCH)
        cl = np.transpose(cl, (0, 1, 3, 2)).reshape(BSH, 16 * NCH, 2, 2)
        logM_all.append(cl - corr)
    mats = np.concatenate(logM_all, axis=0)       # [64, 512, 2, 2]

    def compose(Bm, Am):
        s = Bm[..., :, :, None] + Am[..., None, :, :]
        return _lse(s, axis=-2)

    while mats.shape[1] > 1:
        n = mats.shape[1]
        if n % 2:
            last = mats[:, -1:]
            mats = compose(mats[:, 1::2], mats[:, 0:-1:2])
            mats = np.concatenate([mats, last], axis=1)
        else:
            mats = compose(mats[:, 1::2], mats[:, 0::2])
    den = float(mats[:, 0, 0, 0].sum()) + B * fs
    return np.float32(num - den)


def _lse(x, axis):
    m = np.max(x, axis=axis, keepdims=True)
    m = np.where(np.isfinite(m), m, 0.0)
    out = np.squeeze(m, axis) + np.log(np.sum(np.exp(x - m), axis=axis))
    return out


def kernel(log_probs, den_scores, input_lens, labels):
    nc = _get_program()
    log_probs = np.asarray(log_probs)
    den_scores = np.asarray(den_scores)
    in_maps = _make_in_maps(
        log_probs, den_scores,
        np.asarray(input_lens), np.asarray(labels),
    )
    res = run_bass_kernel_spmd(nc, in_maps, core_ids=list(range(NCORES)))
    return _combine_host(res.results, den_scores)
